# revision 20
# baseline (speedup 1.0000x reference)
"""Trainium2 Bass kernel for the DeformableSDFHead MLP.

Network (per point n, 16 bones k):
  x_k = [xyz3_k (3) | gl (48)]            gl shared per group of 4096 points
  h0  = relu(W0_k x_k + b0_k)             51 -> 64
  h_{l+1} = relu(Wmid_{k,l} h_l + bmid + h_l)   l = 0..6  (residual)
  latent = pre-residual out of l=6
  y = tanh(concat_k(latent_k) . Wf + bf)

Restructuring (all folds done host-side in numpy):
  * gl part of layer 0 folded into a per-(bone, group) bias beff.
  * residual folded into weights: W' = Wmid + I for l=0..5.
  * layer l=6 + final projection folded into a per-bone matvec:
      y = tanh(sum_k v_k . h6_k + c),  v_k = Wmid_{k,6}^T Wf_k.

Mapping: data-parallel over 8 cores (8192 points each). Per core, bones are
packed 4-at-a-time into the 128x128 PE array via tile_position (2x2 grid of
64x64 tiles), activations fp16, psum fp32, relu+bias evacuation split across
ScalarE (activation Relu w/ bias) and VectorE (tensor_scalar add+max).
"""

import numpy as np

import concourse.bacc as bacc
import concourse.bass as bass
import concourse.mybir as mybir
from concourse.tile import TileContext
from concourse.bass_utils import run_bass_kernel_spmd

NUM_BONES = 16
HID = 64
JOINT_IDX = np.array([0, 1, 2, 3, 5, 6, 7, 9, 10, 11, 13, 14, 15, 17, 18, 19])

NCORES = 8
N = 65536
NS = N // NCORES       # 8192 points per core
SG = 2048              # supergroup (points held in SBUF per pipeline stage)
NSG = NS // SG         # 4
F = 512                # matmul free-dim chunk (one psum bank)
RF = 1024              # relu op free dim (2 psum banks)

FP16 = mybir.dt.float16
FP32 = mybir.dt.float32

_SIGMA = [[(2 * p, 2 * p + 1) for p in range(8)]]
for _l in range(6):
    _SIGMA.append([_SIGMA[-1][p] if p % 2 == 0 else _SIGMA[-1][p][::-1]
                   for p in range(8)])


def _host_prep(xyz, joints, W0, b0, Wmid, bmid, Wf, bf):
    f32 = np.float32
    B = joints.shape[0]
    gl = joints[:, JOINT_IDX, :]
    gl = (gl - gl[:, :1, :]).reshape(B, -1).astype(f32)

    W0a = W0[:, :, 0:3].astype(f32)
    W0b = W0[:, :, 3:].astype(f32)
    beff = b0[:, None, :].astype(f32) + np.einsum('gi,koi->kgo', gl, W0b)

    I = np.eye(HID, dtype=f32)
    Wm_f = Wmid[:, :6].astype(f32) + I[None, None]

    Wf_k = Wf.reshape(NUM_BONES, HID).astype(f32)
    v = np.einsum('koi,ko->ki', Wmid[:, 6].astype(f32), Wf_k)
    c = float(np.sum(bmid[:, 6].astype(f32) * Wf_k) + bf[0])

    wm = np.zeros((128, 6 * 8 * 64), dtype=f32)
    bm = np.zeros((128, 48), dtype=f32)
    for l in range(6):
        for p in range(8):
            col = (l * 8 + p) * 64
            blo, bhi = _SIGMA[l][p]
            wm[0:64, col:col + 64] = Wm_f[blo, l].T
            wm[64:128, col:col + 64] = Wm_f[bhi, l].T
            olo, ohi = _SIGMA[l + 1][p]
            bm[0:64, l * 8 + p] = bmid[olo, l]
            bm[64:128, l * 8 + p] = bmid[ohi, l]

    w0 = np.zeros((128, 256), dtype=f32)
    for g in range(4):
        for j in range(4):
            w0[32 * j:32 * j + 3, 64 * g:64 * (g + 1)] = W0a[4 * g + j].T

    vt = np.zeros((128, 8 * 32), dtype=f32)
    for p in range(8):
        blo, bhi = _SIGMA[6][p]
        vt[0:64, 32 * p:32 * p + 32] = v[blo][:, None]
        vt[64:128, 32 * p:32 * p + 32] = v[bhi][:, None]

    xyzf = xyz.astype(f32)
    in_maps = []
    for core in range(NCORES):
        n0 = core * NS
        x3 = np.zeros((4, 12, NS), dtype=np.float16)
        for g in range(4):
            for j in range(4):
                b_ = 4 * g + j
                x3[g, 3 * j:3 * j + 3, :] = (
                    xyzf[n0:n0 + NS, 3 * (b_ + 1):3 * (b_ + 1) + 3].T.astype(np.float16))
        b0e = np.zeros((128, 16), dtype=f32)
        for p in range(8):
            blo, bhi = _SIGMA[0][p]
            for gi in range(2):
                grp = 2 * core + gi
                b0e[0:64, p * 2 + gi] = beff[blo, grp]
                b0e[64:128, p * 2 + gi] = beff[bhi, grp]
        in_maps.append(dict(
            x3=x3,
            w0=w0.astype(np.float16),
            wm=wm.astype(np.float16),
            bm=bm,
            b0e=b0e,
            vt=vt.astype(np.float16),
        ))
    return in_maps, c


_CACHE = {}


def _build():
    nc = bacc.Bacc("TRN2", target_bir_lowering=False)

    x3_h = nc.dram_tensor("x3", [4, 12, NS], FP16, kind="ExternalInput")
    w0_h = nc.dram_tensor("w0", [128, 256], FP16, kind="ExternalInput")
    wm_h = nc.dram_tensor("wm", [128, 6 * 8 * 64], FP16, kind="ExternalInput")
    bm_h = nc.dram_tensor("bm", [128, 48], FP32, kind="ExternalInput")
    b0e_h = nc.dram_tensor("b0e", [128, 16], FP32, kind="ExternalInput")
    vt_h = nc.dram_tensor("vt", [128, 8 * 32], FP16, kind="ExternalInput")
    # rows 0..2: full per-SG sums; rows 3,4: the last SG's matvec is split
    # into two half-chains (p0-3 / p4-7) to shorten the kernel tail — the
    # host adds them.
    out_h = nc.dram_tensor("out", [NSG + 1, 4, F], FP32, kind="ExternalOutput")

    Relu = mybir.ActivationFunctionType.Relu
    Tanh = mybir.ActivationFunctionType.Tanh
    ADD = mybir.AluOpType.add
    MAX = mybir.AluOpType.max

    # relu engine split by measured rates (ACT 1073ns vs DVE 1212ns per op;
    # DVE also runs the 4 matvec copies ~2.8us): ACT gets 239 of 448.
    act_pick = [((i * 239) % 448) < 239 for i in range(448)]

    with TileContext(nc) as tc:
        with (
            tc.tile_pool(name="const", bufs=1) as cpool,
            tc.tile_pool(name="xin", bufs=4) as xpool,
            tc.tile_pool(name="hbuf", bufs=2) as hpool,
            tc.tile_pool(name="outp", bufs=2) as opool,
            tc.tile_pool(name="ps", bufs=4, space="PSUM") as pspool,
        ):
            w0_t = cpool.tile([128, 256], FP16, name="w0t")
            wm_t = cpool.tile([128, 6 * 8 * 64], FP16, name="wmt")
            bm_t = cpool.tile([128, 48], FP32, name="bmt")
            b0e_t = cpool.tile([128, 16], FP32, name="b0et")
            vt_t = cpool.tile([128, 8 * 32], FP16, name="vtt")
            # critical-path DMAs first: L0 consts, then SG0 inputs (issued by
            # load_x below), then per-layer mid weights, vt (needed last) at
            # the end.  One descriptor per x tile (partition-strided AP)
            # instead of 4 keeps the Sync engine's DIRECT2D queue short.
            nc.sync.dma_start(out=w0_t[:, :], in_=w0_h[:, :])
            nc.sync.dma_start(out=b0e_t[:, :], in_=b0e_h[:, :])

            def load_x(sg, interleave=None):
                s0 = sg * SG
                xg = []
                for g in range(4):
                    xt = xpool.tile([128, SG], FP16, name=f"x{g}", tag=f"x{g}")
                    for j in range(4):
                        nc.sync.dma_start(
                            out=xt[32 * j:32 * j + 3, :],
                            in_=x3_h[g, 3 * j:3 * j + 3, s0:s0 + SG])
                    xg.append(xt)
                    if interleave is not None:
                        interleave(g)
                return xg

            relu_i = 0

            def emit_matvec(h6, msg, p0=0, p1=8):
                # 4 col-concurrent accumulation chains (one per cc), p-outer
                # so the chains interleave and overlap in the PE array.
                mv = pspool.tile([128, RF], FP32, name="mv", tag="ps")[:, :F]
                for p in range(p0, p1):
                    for cc in range(4):
                        nc.tensor.matmul(
                            out=mv[32 * cc:32 * cc + 32, :],
                            lhsT=vt_t[:, 32 * p:32 * p + 32],
                            rhs=h6[p][:, cc * F:(cc + 1) * F],
                            start=(p == p0), stop=(p == p1 - 1),
                            tile_position=(0, 32 * cc),
                            skip_group_check=True)
                out_sb = opool.tile([128, F], FP32, name="osb", tag="osb")
                nc.vector.tensor_copy(out_sb[0:97, :], mv[0:97, :])
                ou_v = out_sb.rearrange("(a b) f -> a b f", b=32)[:, 0:1, :]
                nc.sync.dma_start(out=out_h[msg, :, :], in_=ou_v)

            def emit_relu(ps_ap, out_ap, bias_ap):
                nonlocal relu_i
                if act_pick[relu_i % 448]:
                    nc.scalar.activation(out_ap, ps_ap, Relu, bias=bias_ap, scale=1.0)
                else:
                    nc.vector.tensor_scalar(out_ap, ps_ap, bias_ap, 0.0, ADD, MAX)
                relu_i += 1

            def alloc_l0(sg):
                return [hpool.tile([128, SG], FP16, name=f"h{p}_a",
                                   tag=f"h{p}_a") for p in range(8)]

            def emit_l0_group(xg, sg, h_cur, g):
                glocal = sg // 2
                if True:
                    for half in range(2):
                        psA = pspool.tile([128, RF], FP32, name="psA", tag="ps")
                        psB = pspool.tile([128, RF], FP32, name="psB", tag="ps")
                        for ccl in range(2):
                            cc = 2 * half + ccl
                            for j in range(4):
                                ps = psA if j < 2 else psB
                                colh = 64 * (j % 2)
                                nc.tensor.matmul(
                                    out=ps[colh:colh + 64, ccl * F:(ccl + 1) * F],
                                    lhsT=w0_t[32 * j:32 * j + 3, 64 * g:64 * (g + 1)],
                                    rhs=xg[g][32 * j:32 * j + 3, cc * F:(cc + 1) * F],
                                    start=True, stop=True,
                                    tile_position=(32 * j, colh))
                        hs = slice(half * RF, (half + 1) * RF)
                        pA, pB = 2 * g, 2 * g + 1
                        emit_relu(psA[:, :], h_cur[pA][:, hs],
                                  b0e_t[:, pA * 2 + glocal:pA * 2 + glocal + 1])
                        emit_relu(psB[:, :], h_cur[pB][:, hs],
                                  b0e_t[:, pB * 2 + glocal:pB * 2 + glocal + 1])

            def emit_l0(xg, sg):
                h_cur = alloc_l0(sg)
                for g in range(4):
                    emit_l0_group(xg, sg, h_cur, g)
                return h_cur

            def emit_mid(h_cur, inject=None, tail_hook=None):
                for l in range(6):
                    suf = "b" if l % 2 == 0 else "a"
                    h_nxt = [hpool.tile([128, SG], FP16, name=f"h{p}_{suf}",
                                        tag=f"h{p}_{suf}") for p in range(8)]
                    for q in range(4):
                        colA = (l * 8 + 2 * q) * 64
                        colB = (l * 8 + 2 * q + 1) * 64
                        for half in range(2):
                            psA = pspool.tile([128, RF], FP32, name="psA", tag="ps")
                            psB = pspool.tile([128, RF], FP32, name="psB", tag="ps")
                            for ccl in range(2):
                                cc = 2 * half + ccl
                                fs = slice(cc * F, (cc + 1) * F)
                                os_ = slice(ccl * F, (ccl + 1) * F)
                                nc.tensor.matmul(
                                    out=psA[0:64, os_],
                                    lhsT=wm_t[0:64, colA:colA + 64],
                                    rhs=h_cur[2 * q][0:64, fs],
                                    start=True, stop=True)
                                nc.tensor.matmul(
                                    out=psA[64:128, os_],
                                    lhsT=wm_t[64:128, colA:colA + 64],
                                    rhs=h_cur[2 * q][64:128, fs],
                                    start=True, stop=True)
                                nc.tensor.matmul(
                                    out=psB[64:128, os_],
                                    lhsT=wm_t[0:64, colB:colB + 64],
                                    rhs=h_cur[2 * q + 1][0:64, fs],
                                    start=True, stop=True)
                                nc.tensor.matmul(
                                    out=psB[0:64, os_],
                                    lhsT=wm_t[64:128, colB:colB + 64],
                                    rhs=h_cur[2 * q + 1][64:128, fs],
                                    start=True, stop=True)
                            hs = slice(half * RF, (half + 1) * RF)
                            emit_relu(psA[:, :], h_nxt[2 * q][:, hs],
                                      bm_t[:, l * 8 + 2 * q:l * 8 + 2 * q + 1])
                            emit_relu(psB[:, :], h_nxt[2 * q + 1][:, hs],
                                      bm_t[:, l * 8 + 2 * q + 1:l * 8 + 2 * q + 2])
                        # inject the previous SG's matvec chain mid-layer:
                        # both engines then have 2 q-groups of queued relu
                        # work to drain while the serial chain runs on PE.
                        if l == 0 and q == 1 and inject is not None:
                            inject()
                        # interleave the next SG's L0 with the last mid
                        # layer so the L0->mid transition keeps PE lead.
                        if l == 5 and tail_hook is not None:
                            tail_hook(q, h_nxt)
                    h_cur = h_nxt
                return h_cur

            # SG0 inputs + per-layer mid weights (layer 0 first so the first
            # mid layer never waits on the big wm transfer); vt last; then
            # the remaining SGs' inputs (xpool bufs=4 so no ring waits).
            xgs = [None] * NSG
            xgs[0] = load_x(0)
            for l in range(6):
                c0 = l * 512
                nc.sync.dma_start(out=wm_t[:, c0:c0 + 512],
                                  in_=wm_h[:, c0:c0 + 512])
                if l == 0:
                    nc.sync.dma_start(out=bm_t[:, :], in_=bm_h[:, :])
            nc.sync.dma_start(out=vt_t[:, :], in_=vt_h[:, :])
            for sg in range(1, NSG):
                xgs[sg] = load_x(sg)

            # software-pipelined over SGs: L0 of sg+1 is interleaved into
            # mid(sg)'s last layer (tail_hook), and the matvec of sg is
            # deferred into the middle of mid(sg+1)'s first layer so its
            # serial PE chain is covered by queued relu work.
            h_cur = emit_l0(xgs[0], 0)
            pending = None
            for sg in range(NSG):
                if pending is not None:
                    p_h6, p_sg = pending
                    inject = lambda h6=p_h6, s=p_sg: emit_matvec(h6, s)
                else:
                    inject = None
                if sg + 1 < NSG:
                    l0_state = {}

                    def hook(q, _h, x=xgs[sg + 1], s=sg + 1, st=l0_state):
                        if "h" not in st:
                            st["h"] = alloc_l0(s)
                        emit_l0_group(x, s, st["h"], q)
                else:
                    l0_state = None

                    def hook(q, hh):
                        # last SG: matvec as two half-chains riding inside
                        # l=5 — shortens the kernel tail to one half-chain.
                        if q == 1:
                            emit_matvec(hh, NSG - 1, 0, 4)
                        elif q == 3:
                            emit_matvec(hh, NSG, 4, 8)
                h6 = emit_mid(h_cur, inject=inject, tail_hook=hook)
                h_cur = l0_state["h"] if l0_state else None
                pending = (h6, sg)
    nc.finalize()
    return nc


def _gather(results):
    ys = []
    for r in results:
        o = r["out"]
        ys.append(o[0:NSG - 1].reshape(-1))
        ys.append((o[NSG - 1] + o[NSG]).reshape(-1))
    return np.concatenate(ys)


def kernel(xyz, joints, W0, b0, Wmid, bmid, Wf, bf):
    in_maps, c = _host_prep(xyz, joints, W0, b0, Wmid, bmid, Wf, bf)
    key = "nc"
    if key not in _CACHE:
        _CACHE[key] = _build()
    nc = _CACHE[key]
    res = run_bass_kernel_spmd(nc, in_maps, core_ids=list(range(NCORES)))
    return np.tanh(_gather(res.results) + c).reshape(N, 1).astype(np.float32)

